# revision 1
# baseline (speedup 1.0000x reference)
"""DBML loss on 8 Trainium2 NeuronCores (Bass/Tile, SPMD row-parallel).

Strategy
--------
Rows are sorted by label on the host so each 128-row chunk's same-label
columns fall inside a narrow W-wide "band". Per core (512 rows):

  *  S~ = sim - 4*[same label]  is computed directly as ONE fp32r matmul of
     label-augmented features (feats ++ +-2*onehot, contraction dim 640).
     Non-same entries of S~ equal sim bit-exactly, so every full-row
     ("negative side") quantity reads S~ alone, and sames are auto-excluded
     from the threshold selection (they sit at sim-4 < -3).
  *  Negative-side sums use the relu factorization around the per-row
     threshold t = min_pos - margin:
        v          = relu(S~ - t)
        n_neg      = sum 1{S~ > t}
        sum_sel S~   = sum v + t*n_neg
        sum_sel S~^2 = sum v^2 + t*(2*sum_sel S~ - t*n_neg)
        fn_sum     = e^(2t-1.2) * (sum exp(2v) - B + n_neg)
     so everything rides on single-tensor DVE passes (2x mode) and ACT
     activations with free accum_out reductions.
  *  The positive side (same-label pairs) only needs the band: a tiny
     augmented [128, W] matmul (min_pos = rowmin straight from PSUM) plus
     host masks (pos = 4*(same minus diag)) that are only needed late.
  *  sigma_all is recovered algebraically:  S2 = sum(S~^2) + 8*sum_same(sim)
     - 16*n_same,  sigma_all = S2 - S1^2/B.

All per-row statistics accumulate into [128, #chunks]-wide tiles; one
vectorized finalize computes the 512 per-row losses per core. The host sums
the 4096 rows and divides by B.
"""

import numpy as np

B = 4096
D = 512
NCLS = 100
NCORES = 8
RPC = B // NCORES          # rows per core = 512
P = 128                    # partitions
MCH = RPC // P             # m-chunks per core = 4
KF = D // P                # feats k-chunks = 4
KA = 5                     # augmented k-chunks (640 = 5*128)
DAUG = KA * P              # 640
HALF = 1024                # free-dim span for elementwise tiles
NBANK = 512                # psum chunk width
NB = HALF // NBANK         # psum chunks per span = 2
NH = B // HALF             # spans = 4
NS = B // NBANK            # 512-col slices of augT = 8

POS_A, POS_B = 1.0, 0.5
NEG_A, NEG_B = 0.6, 0.5
MARGIN, WEIGHT = 0.1, 0.5

_CACHE = {}


def _build_program(W):
    import concourse.bacc as bacc
    import concourse.mybir as mybir
    import concourse.tile as tile
    from contextlib import ExitStack

    f32 = mybir.dt.float32
    bf16 = mybir.dt.bfloat16
    f32r = mybir.dt.float32r
    Alu = mybir.AluOpType
    Act = mybir.ActivationFunctionType
    AX = mybir.AxisListType

    nc = bacc.Bacc(
        "TRN2", target_bir_lowering=False, debug=False, num_devices=NCORES
    )

    # ---- DRAM I/O (per-core) ----
    augT_d = nc.dram_tensor("augT", [KA, P, B], f32r, kind="ExternalInput").ap()
    augMyT_d = nc.dram_tensor("augMyT", [KA, P, RPC], f32r, kind="ExternalInput").ap()
    bandT_d = nc.dram_tensor("bandT", [KA, P, MCH * W], f32r, kind="ExternalInput").ap()
    posB_d = nc.dram_tensor("posB", [MCH, P, W], f32, kind="ExternalInput").ap()
    sameB_d = nc.dram_tensor("sameB", [MCH, P, W], f32, kind="ExternalInput").ap()
    nsame_d = nc.dram_tensor("nsame", [P, MCH], f32, kind="ExternalInput").ap()
    loss_d = nc.dram_tensor("loss", [P, MCH], f32, kind="ExternalOutput").ap()

    with tile.TileContext(nc) as tc, ExitStack() as ctx:
        p_aug = ctx.enter_context(tc.tile_pool(name="aug", bufs=1))
        p_augmy = ctx.enter_context(tc.tile_pool(name="augmy", bufs=1))
        p_bandt = ctx.enter_context(tc.tile_pool(name="bandt", bufs=1))
        p_mask = ctx.enter_context(tc.tile_pool(name="mask", bufs=1))
        p_sh = ctx.enter_context(tc.tile_pool(name="sh", bufs=3))
        p_v = ctx.enter_context(tc.tile_pool(name="v", bufs=2))
        p_dvd = ctx.enter_context(tc.tile_pool(name="dvd", bufs=2))
        p_dva = ctx.enter_context(tc.tile_pool(name="dva", bufs=1))
        p_band = ctx.enter_context(tc.tile_pool(name="band", bufs=2))
        p_stat = ctx.enter_context(tc.tile_pool(name="stat", bufs=1))
        p_ps = ctx.enter_context(tc.tile_pool(name="ps", bufs=4, space="PSUM"))
        p_psb = ctx.enter_context(tc.tile_pool(name="psb", bufs=2, space="PSUM"))

        # ---- DMA order: band operands + my-side first (they gate the
        # thresholds), then column-sliced augT, masks last (used late) ----
        bandt = []
        for k in range(KA):
            t = p_bandt.tile([P, MCH * W], f32r, tag=f"bandt{k}", name=f"bandt{k}")
            nc.sync.dma_start(t[:], bandT_d[k])
            bandt.append(t)
        augmy = []
        for k in range(KA):
            t = p_augmy.tile([P, RPC], f32r, tag=f"augmy{k}", name=f"augmy{k}")
            nc.sync.dma_start(t[:], augMyT_d[k])
            augmy.append(t)
        nsamem = p_stat.tile([P, MCH], f32, tag="nsamem")
        nc.sync.dma_start(nsamem[:], nsame_d)
        aug = [[None] * NS for _ in range(KA)]
        for j in range(NS):
            for k in range(KA):
                t = p_aug.tile(
                    [P, NBANK], f32r, tag=f"aug{k}_{j}", name=f"aug{k}_{j}"
                )
                nc.sync.dma_start(t[:], augT_d[k, :, j * NBANK : (j + 1) * NBANK])
                aug[k][j] = t
        masks = []
        for m in range(MCH):
            posm = p_mask.tile([P, W], f32, tag=f"posm{m}", name=f"posm{m}")
            nc.sync.dma_start(posm[:], posB_d[m])
            samem = p_mask.tile([P, W], f32, tag=f"samem{m}", name=f"samem{m}")
            nc.sync.dma_start(samem[:], sameB_d[m])
            masks.append((posm, samem))

        # activation bias constants (must be APs for non-Copy funcs)
        bias_e2 = p_stat.tile([P, 1], f32, tag="bias_e2")
        nc.gpsimd.memset(bias_e2[:], -1.2)
        bias_e1 = p_stat.tile([P, 1], f32, tag="bias_e1")
        nc.gpsimd.memset(bias_e1[:], 2.0)

        # ---- wide accumulators (written via accum_out slices, read in finalize)
        a_sst = p_stat.tile([P, MCH * NH * NB], f32, tag="a_sst")   # [P,32]
        a_sq = p_stat.tile([P, MCH * NH], f32, tag="a_sq")
        a_nneg = p_stat.tile([P, MCH * NH], f32, tag="a_nneg")
        a_mneg = p_stat.tile([P, MCH * NH], f32, tag="a_mneg")
        a_relu = p_stat.tile([P, MCH * NH], f32, tag="a_relu")
        a_v2 = p_stat.tile([P, MCH * NH], f32, tag="a_v2")
        a_e2v = p_stat.tile([P, MCH * NH], f32, tag="a_e2v")
        a_npos = p_stat.tile([P, MCH], f32, tag="a_npos")
        a_fp = p_stat.tile([P, MCH], f32, tag="a_fp")
        a_ps = p_stat.tile([P, MCH], f32, tag="a_ps")
        a_ps2 = p_stat.tile([P, MCH], f32, tag="a_ps2")
        a_sames = p_stat.tile([P, MCH], f32, tag="a_sames")
        a_mpos = p_stat.tile([P, MCH], f32, tag="a_mpos")
        a_t = p_stat.tile([P, MCH], f32, tag="a_t")   # threshold t per (row, m)

        # ---------- band phase: augmented band matmul; min_pos from PSUM ----
        sb_tiles = {}
        for m in range(MCH):
            ms = slice(m * P, (m + 1) * P)
            psb = p_psb.tile([P, W], f32, tag="psb")
            for k in range(KA):
                nc.tensor.matmul(
                    psb[:],
                    augmy[k][:, ms],
                    bandt[k][:, m * W : (m + 1) * W],
                    start=(k == 0),
                    stop=(k == KA - 1),
                )
            # min over band of S~band = min_pos - 4 (sames incl diag sit low)
            nc.vector.tensor_reduce(
                a_mpos[:, m : m + 1], psb[:], axis=AX.X, op=Alu.min
            )
            # threshold t = (minpos - 4) + 4 - 0.1
            nc.vector.tensor_scalar(
                a_t[:, m : m + 1], a_mpos[:, m : m + 1], 3.9, None, Alu.add
            )
            sb = p_band.tile([P, W], f32, tag=f"sb{m}", name=f"sb{m}")
            nc.scalar.activation(sb[:], psb[:], Act.Copy)
            sb_tiles[m] = sb

        # ---------- full-row side, span-outer so early column slices feed
        # all four row chunks before later slices arrive ----------
        for h in range(NH):
            for m in range(MCH):
                ms = slice(m * P, (m + 1) * P)
                hx = m * NH + h
                sh = p_sh.tile([P, HALF], f32, tag="sh")
                for nb in range(NB):
                    ps = p_ps.tile([P, NBANK], f32, tag="ps")
                    for k in range(KA):
                        nc.tensor.matmul(
                            ps[:],
                            augmy[k][:, ms],
                            aug[k][h * NB + nb][:],
                            start=(k == 0),
                            stop=(k == KA - 1),
                        )
                    nc.scalar.activation(
                        sh[:, nb * NBANK : (nb + 1) * NBANK],
                        ps[:],
                        Act.Copy,
                        accum_out=a_sst[:, hx * NB + nb : hx * NB + nb + 1],
                    )

                # sum(S~^2)  (DVE STT, dead out)
                d1 = p_dvd.tile([P, HALF], bf16, tag="dvd")
                nc.vector.scalar_tensor_tensor(
                    out=d1[:],
                    in0=sh[:],
                    scalar=0.0,
                    in1=sh[:],
                    op0=Alu.add,
                    op1=Alu.mult,
                    accum_out=a_sq[:, hx : hx + 1],
                )
                # n_neg  (dead out)
                d2 = p_dvd.tile([P, HALF], bf16, tag="dvd")
                nc.vector.tensor_scalar(
                    d2[:],
                    sh[:],
                    a_t[:, m : m + 1],
                    None,
                    Alu.is_gt,
                    Alu.add,
                    accum_out=a_nneg[:, hx : hx + 1],
                )
                # row max of S~  (dead out, max-accum)
                d3 = p_dvd.tile([P, HALF], bf16, tag="dvd")
                nc.vector.tensor_scalar(
                    d3[:],
                    sh[:],
                    0.0,
                    None,
                    Alu.add,
                    Alu.max,
                    accum_out=a_mneg[:, hx : hx + 1],
                )
                # v = relu(S~ - t)   (dual-op TS, no accum)
                v = p_v.tile([P, HALF], f32, tag="v")
                nc.vector.tensor_scalar(
                    v[:], sh[:], a_t[:, m : m + 1], 0.0, Alu.subtract, Alu.max
                )
                # sum v  (dead out)
                d4 = p_dvd.tile([P, HALF], bf16, tag="dvd")
                nc.vector.tensor_scalar(
                    d4[:],
                    v[:],
                    0.0,
                    None,
                    Alu.add,
                    Alu.add,
                    accum_out=a_relu[:, hx : hx + 1],
                )
                # sum v^2  (ACT)
                a1 = p_dva.tile([P, HALF], bf16, tag="dva")
                nc.scalar.activation(
                    a1[:], v[:], Act.Square, accum_out=a_v2[:, hx : hx + 1]
                )
                # sum exp(2v)  (ACT)
                a2 = p_dva.tile([P, HALF], bf16, tag="dva")
                nc.scalar.activation(
                    a2[:],
                    v[:],
                    Act.Exp,
                    bias=0.0,
                    scale=2.0,
                    accum_out=a_e2v[:, hx : hx + 1],
                )

                # ---- band selection, interleaved into the last span ----
                if h == NH - 1:
                    sb = sb_tiles[m]
                    posm, samem = masks[m]
                    mneg01 = p_stat.tile(
                        [P, 1], f32, tag=f"mneg01{m}", name=f"mneg01{m}"
                    )
                    nc.vector.tensor_reduce(
                        mneg01[:],
                        a_mneg[:, m * NH : (m + 1) * NH],
                        axis=AX.X,
                        op=Alu.max,
                    )
                    nc.vector.tensor_scalar(
                        mneg01[:], mneg01[:], 0.1, None, Alu.add
                    )
                    # X = sb + posm: true sim at pos entries, sb elsewhere
                    x = p_band.tile([P, W], f32, tag="x")
                    nc.vector.tensor_tensor(x[:], sb[:], posm[:], Alu.add)
                    # psel = 4 * pos01 * (sim < mneg01)
                    psel = p_band.tile([P, W], f32, tag="psel")
                    nc.vector.scalar_tensor_tensor(
                        out=psel[:],
                        in0=x[:],
                        scalar=mneg01[:],
                        in1=posm[:],
                        op0=Alu.is_lt,
                        op1=Alu.mult,
                        accum_out=a_npos[:, m : m + 1],
                    )
                    e1b = p_band.tile([P, W], f32, tag="e1b")
                    nc.scalar.activation(
                        e1b[:], x[:], Act.Exp, bias=bias_e1[:], scale=-2.0
                    )
                    scrb2 = p_band.tile([P, W], f32, tag="scrb")
                    nc.vector.scalar_tensor_tensor(
                        out=scrb2[:],
                        in0=e1b[:],
                        scalar=0.0,
                        in1=psel[:],
                        op0=Alu.add,
                        op1=Alu.mult,
                        accum_out=a_fp[:, m : m + 1],
                    )
                    psb1 = p_band.tile([P, W], f32, tag="psb1")
                    nc.vector.scalar_tensor_tensor(
                        out=psb1[:],
                        in0=psel[:],
                        scalar=0.0,
                        in1=x[:],
                        op0=Alu.add,
                        op1=Alu.mult,
                        accum_out=a_ps[:, m : m + 1],
                    )
                    psb2 = p_band.tile([P, W], f32, tag="psb1")
                    nc.vector.scalar_tensor_tensor(
                        out=psb2[:],
                        in0=psb1[:],
                        scalar=0.0,
                        in1=x[:],
                        op0=Alu.add,
                        op1=Alu.mult,
                        accum_out=a_ps2[:, m : m + 1],
                    )
                    # sum_same sim - 4 (diag sits at sim-4 in X)
                    psb3 = p_band.tile([P, W], f32, tag="psb1")
                    nc.vector.scalar_tensor_tensor(
                        out=psb3[:],
                        in0=samem[:],
                        scalar=0.0,
                        in1=x[:],
                        op0=Alu.add,
                        op1=Alu.mult,
                        accum_out=a_sames[:, m : m + 1],
                    )

        # ---------- vectorized finalize over [P, MCH] ----------
        p_fin = ctx.enter_context(tc.tile_pool(name="fin", bufs=1))

        def fin(tag):
            return p_fin.tile([P, MCH], f32, tag=tag, name=tag)

        def red(dst, src, width, op):
            nc.vector.tensor_reduce(
                dst, src.rearrange("p (m w) -> p m w", w=width), axis=AX.X, op=op
            )

        sst = fin("sst")
        red(sst[:], a_sst[:], NH * NB, Alu.add)
        sumsq = fin("sumsq")
        red(sumsq[:], a_sq[:], NH, Alu.add)
        nneg = fin("nneg")
        red(nneg[:], a_nneg[:], NH, Alu.add)
        relu_s = fin("relu_s")
        red(relu_s[:], a_relu[:], NH, Alu.add)
        v2_s = fin("v2_s")
        red(v2_s[:], a_v2[:], NH, Alu.add)
        e2v_s = fin("e2v_s")
        red(e2v_s[:], a_e2v[:], NH, Alu.add)

        # rescale the 4-weighted pos-side sums
        npos = fin("npos")
        nc.vector.tensor_scalar(npos[:], a_npos[:], 0.25, None, Alu.mult)
        fpsum = fin("fpsum")
        nc.vector.tensor_scalar(fpsum[:], a_fp[:], 0.25, None, Alu.mult)
        pS = fin("pS")
        nc.vector.tensor_scalar(pS[:], a_ps[:], 0.25, None, Alu.mult)
        pS2 = fin("pS2")
        nc.vector.tensor_scalar(pS2[:], a_ps2[:], 0.25, None, Alu.mult)

        # neg-side recoveries from the relu factorization
        tn = fin("tn")
        nc.vector.tensor_tensor(tn[:], a_t[:], nneg[:], Alu.mult)   # t*n_neg
        c2s = fin("c2s")
        nc.vector.tensor_tensor(c2s[:], relu_s[:], tn[:], Alu.add)
        u2 = fin("u2")
        nc.vector.scalar_tensor_tensor(
            u2[:], c2s[:], 2.0, tn[:], Alu.mult, Alu.subtract
        )  # 2*c2s - t*n_neg
        u3 = fin("u3")
        nc.vector.tensor_tensor(u3[:], a_t[:], u2[:], Alu.mult)
        c2s2 = fin("c2s2")
        nc.vector.tensor_tensor(c2s2[:], v2_s[:], u3[:], Alu.add)
        # fn_sum = exp(2t-1.2) * (e2v_s - B + n_neg)
        eT = fin("eT")
        nc.scalar.activation(eT[:], a_t[:], Act.Exp, bias=bias_e2[:], scale=2.0)
        q = fin("q")
        nc.vector.scalar_tensor_tensor(
            q[:], e2v_s[:], -float(B), nneg[:], Alu.add, Alu.add
        )
        fnsum = fin("fnsum")
        nc.vector.tensor_tensor(fnsum[:], eT[:], q[:], Alu.mult)

        # S1 = sum(S~) + 4*nsame
        # S2 = sum(S~^2) + 8*(sum_same sim) - 16*nsame, with the band's
        # a_sames = sum_same sim - 4  =>  S2 = sumsq + 8*a_sames + 32 - 16*nsame
        s1 = fin("s1")
        nc.vector.scalar_tensor_tensor(
            s1[:], nsamem[:], 4.0, sst[:], Alu.mult, Alu.add
        )
        t8 = fin("t8")
        nc.vector.scalar_tensor_tensor(
            t8[:], a_sames[:], 8.0, sumsq[:], Alu.mult, Alu.add
        )
        nc.vector.tensor_scalar(t8[:], t8[:], 32.0, None, Alu.add)
        s2 = fin("s2")
        nc.vector.scalar_tensor_tensor(
            s2[:], nsamem[:], -16.0, t8[:], Alu.mult, Alu.add
        )
        mean_all = fin("mean_all")
        nc.vector.tensor_scalar(mean_all[:], s1[:], 1.0 / B, None, Alu.mult)
        s1m = fin("s1m")
        nc.vector.tensor_tensor(s1m[:], s1[:], mean_all[:], Alu.mult)
        sigma_all = fin("sigma_all")
        nc.vector.tensor_tensor(sigma_all[:], s2[:], s1m[:], Alu.subtract)

        cnt = fin("cnt")
        nc.vector.tensor_tensor(cnt[:], npos[:], nneg[:], Alu.add)
        nc.vector.tensor_scalar(cnt[:], cnt[:], 1.0, None, Alu.max)
        rec = fin("rec")
        nc.vector.reciprocal(rec[:], cnt[:])
        sels = fin("sels")
        nc.vector.tensor_tensor(sels[:], pS[:], c2s[:], Alu.add)
        sels2 = fin("sels2")
        nc.vector.tensor_tensor(sels2[:], pS2[:], c2s2[:], Alu.add)
        mean_sel = fin("mean_sel")
        nc.vector.tensor_tensor(mean_sel[:], sels[:], rec[:], Alu.mult)
        ss2 = fin("ss2")
        nc.vector.tensor_tensor(ss2[:], sels2[:], rec[:], Alu.mult)
        msq = fin("msq")
        nc.vector.tensor_tensor(msq[:], mean_sel[:], mean_sel[:], Alu.mult)
        sigma_sel = fin("sigma_sel")
        nc.vector.tensor_tensor(sigma_sel[:], ss2[:], msq[:], Alu.subtract)

        fp1 = fin("fp1")
        nc.vector.tensor_scalar(fp1[:], fpsum[:], 1.0, None, Alu.add)
        fn1 = fin("fn1")
        nc.vector.tensor_scalar(fn1[:], fnsum[:], 1.0, None, Alu.add)
        # invalid rows can produce junk (even <= 0) fn1; clamp before Ln,
        # the valid-mask zeroes them anyway
        nc.vector.tensor_scalar(fn1[:], fn1[:], 1e-20, None, Alu.max)
        logfp = fin("logfp")
        nc.scalar.activation(logfp[:], fp1[:], Act.Ln)
        logfn = fin("logfn")
        nc.scalar.activation(logfn[:], fn1[:], Act.Ln)

        dm = fin("dm")
        nc.vector.tensor_tensor(dm[:], mean_all[:], mean_sel[:], Alu.subtract)
        dma = fin("dma")
        nc.scalar.activation(dma[:], dm[:], Act.Abs)
        dsg = fin("dsg")
        nc.vector.tensor_tensor(dsg[:], sigma_all[:], sigma_sel[:], Alu.subtract)
        dsga = fin("dsga")
        nc.scalar.activation(dsga[:], dsg[:], Act.Abs)
        dsum = fin("dsum")
        nc.vector.tensor_tensor(dsum[:], dma[:], dsga[:], Alu.add)
        logs = fin("logs")
        nc.vector.tensor_tensor(logs[:], logfp[:], logfn[:], Alu.add)
        loss_i = fin("loss_i")
        nc.vector.scalar_tensor_tensor(
            loss_i[:], dsum[:], WEIGHT, logs[:], Alu.mult, Alu.add
        )

        vmin = fin("vmin")
        nc.vector.tensor_tensor(vmin[:], npos[:], nneg[:], Alu.min)
        valid = fin("valid")
        nc.vector.tensor_scalar(valid[:], vmin[:], 0.5, None, Alu.is_ge)
        lossm = fin("lossm")
        nc.vector.tensor_tensor(lossm[:], loss_i[:], valid[:], Alu.mult)

        nc.sync.dma_start(loss_d, lossm[:])

    nc.compile()
    return nc


def _host_prep(feats, labels, W):
    feats = np.ascontiguousarray(np.asarray(feats, dtype=np.float32))
    labels = np.asarray(labels).astype(np.int64)
    order = np.argsort(labels, kind="stable")
    feats_s = np.ascontiguousarray(feats[order])
    lab_s = labels[order]
    cnt = np.bincount(lab_s, minlength=NCLS)
    cum = np.concatenate([[0], np.cumsum(cnt)])
    nsame_all = cnt[lab_s].astype(np.float32)

    augT = np.zeros((KA, P, B), np.float32)
    augT.reshape(DAUG, B)[:D] = feats_s.T
    oh2 = np.zeros((NCLS, B), np.float32)
    oh2[lab_s, np.arange(B)] = 2.0
    augT.reshape(DAUG, B)[D : D + NCLS] = oh2
    augT2 = augT.reshape(DAUG, B)

    in_maps = []
    for c in range(NCORES):
        c0 = c * RPC
        augMyT = np.ascontiguousarray(augT[:, :, c0 : c0 + RPC])
        augMyT.reshape(DAUG, RPC)[D : D + NCLS] *= -1.0

        bandT = np.zeros((KA, P, MCH * W), np.float32)
        posB = np.zeros((MCH, P, W), np.float32)
        sameB = np.zeros((MCH, P, W), np.float32)
        nsame = np.zeros((P, MCH), np.float32)
        for m in range(MCH):
            r0 = c0 + m * P
            lo = cum[lab_s[r0]]
            hi = cum[lab_s[r0 + P - 1] + 1]
            if hi - lo > W:
                raise ValueError(f"band too wide: {hi - lo} > {W}")
            u0 = int(min(lo, B - W))
            bandT[:, :, m * W : (m + 1) * W] = augT2[:, u0 : u0 + W].reshape(
                KA, P, W
            )
            labb = lab_s[u0 : u0 + W]
            mylab = lab_s[r0 : r0 + P]
            same = (labb[None, :] == mylab[:, None]).astype(np.float32)
            gcol = np.arange(u0, u0 + W)
            diag = (gcol[None, :] == np.arange(r0, r0 + P)[:, None]).astype(np.float32)
            sameB[m] = same
            posB[m] = 4.0 * same * (1.0 - diag)
            nsame[:, m] = nsame_all[r0 : r0 + P]
        in_maps.append(
            {
                "augT": augT,
                "augMyT": augMyT,
                "bandT": bandT,
                "posB": posB,
                "sameB": sameB,
                "nsame": nsame,
            }
        )
    return in_maps


def kernel(feats, labels):
    from concourse.bass_utils import run_bass_kernel_spmd

    W = 256
    in_maps = _host_prep(feats, labels, W)
    if W not in _CACHE:
        _CACHE[W] = _build_program(W)
    nc = _CACHE[W]
    res = run_bass_kernel_spmd(nc, in_maps, list(range(NCORES)))
    total = np.float64(0.0)
    for c in range(NCORES):
        total += np.asarray(res.results[c]["loss"], dtype=np.float64).sum()
    return np.float32(total / B)

